# revision 12
# baseline (speedup 1.0000x reference)
"""Complex-valued multi-head attention on 8 Trainium2 NeuronCores.

Sharding: batch(2) x head-pairs(4) -> 8 cores; each core runs one batch
element and 2 heads end-to-end (QKV proj -> complex scores -> |s| softmax
-> AV -> partial W_O), host sums the W_O partials over the 4 cores of each
batch element (tensor-parallel reduce) and transposes to the output layout.

All matmuls run in float32r (single-pass fp32, ~1e-3 relerr) except the
post-softmax AV/rowsum path which is fp16. Softmax magnitude uses
|s|/8 = exp(0.5*ln(re^2+im^2) - ln 8) so every ScalarE op (square, ln,
exp, copy) stays inside the one loaded ACT table set.
"""
import sys

sys.path.insert(0, "/opt/trn_rl_repo")

import numpy as np

B, NQ, NK, R = 2, 2048, 2048, 512
H, DK, DV = 8, 64, 64
NCORES = 8
NCC = 8          # n-chunks for projection streaming (2048/256)
NCW = 256        # projection n-chunk width
QC = 4           # q-chunks in attention (2048/512)
QCW = 512
KT = 16          # k-tiles (2048/128)

_CACHE = {}
DEBUG = False


def _build_nc():
    import concourse.bass as bass
    import concourse.tile as tile
    from concourse.tile import add_dep_helper
    from concourse import bacc, mybir

    f32 = mybir.dt.float32
    f32r = mybir.dt.float32r
    f16 = mybir.dt.float16
    ALU = mybir.AluOpType
    AF = mybir.ActivationFunctionType

    nc = bacc.Bacc("TRN2", target_bir_lowering=False, debug=False,
                   num_devices=NCORES)

    xpack_e = nc.dram_tensor("xpack", [NCC, 24, 128, NCW], f32r,
                             kind="ExternalInput")
    wpack_e = nc.dram_tensor("wpack", [128, 36 * 128], f32r,
                             kind="ExternalInput")
    wopack_e = nc.dram_tensor("wopack", [128, 3 * 512], f32r,
                              kind="ExternalInput")
    onesr_e = nc.dram_tensor("onesr", [1, 128], f32r, kind="ExternalInput")
    ident_e = nc.dram_tensor("ident", [128, 128], f16, kind="ExternalInput")
    dbg_es = {}
    if DEBUG:
        for nm, w in (("d_qT_re", NQ), ("d_kT_re", NK), ("d_kT_imn", NK),
                      ("d_vT16_h0", NK), ("d_v16_h0", NK), ("d_oT_re", NQ),
                      ("d_p00", 1024), ("d_sre00", 1024), ("d_rs0", 512)):
            dbg_es[nm] = nc.dram_tensor(nm, [128, w], f32,
                                        kind="ExternalOutput")
    ore_e = nc.dram_tensor("out_re", [512, NQ], f32, kind="ExternalOutput")
    oim_e = nc.dram_tensor("out_im", [512, NQ], f32, kind="ExternalOutput")

    with tile.TileContext(nc) as tc:
      with nc.allow_low_precision(reason="fp16 softmax path"):
        with tc.tile_pool(name="pers", bufs=1) as pers, \
             tc.tile_pool(name="work", bufs=2) as work, \
             tc.tile_pool(name="pwork", bufs=3) as pwork, \
             tc.tile_pool(name="psA", bufs=1, space="PSUM") as psA:

            # ---- constants ----
            wp = pers.tile([128, 36 * 128], f32r, tag="wp")
            nc.sync.dma_start(wp[:], wpack_e[:])
            wop = pers.tile([128, 3 * 512], f32r, tag="wop")
            nc.sync.dma_start(wop[:], wopack_e[:])
            ones_row = pers.tile([1, 128], f32r, tag="ones_row")
            nc.sync.dma_start(ones_row[:], onesr_e[:])
            ident16 = pers.tile([128, 128], f16, tag="ident16")
            nc.sync.dma_start(ident16[:], ident_e[:])
            ones16 = pers.tile([128, 1], f16, tag="ones16")
            nc.vector.memset(ones16[:], 1.0)
            eb_exp = pers.tile([128, 1], f32, tag="eb_exp")
            nc.vector.memset(eb_exp[:], -1.5)          # exp(mag - 1.5)
            eb_mag = pers.tile([128, 1], f32, tag="eb_mag")
            nc.vector.memset(eb_mag[:], -float(np.log(8.0)))  # mag=exp(.5ln u - ln8)

            # ---- projections: qT,kT [hd=128, n=2048] f32r ; vT fp16 ----
            qT_re = pers.tile([128, NQ], f32r, tag="qT_re")
            qT_im = pers.tile([128, NQ], f32r, tag="qT_im")
            kT_re = pers.tile([128, NK], f32r, tag="kT_re")
            kT_im = pers.tile([128, NK], f32r, tag="kT_im")
            kT_imn = pers.tile([128, NK], f32r, tag="kT_imn")
            vT16_h = [pers.tile([128, NK], f16, tag=f"vT16_h{h}",
                                name=f"vT16_h{h}") for h in (0, 1)]

            # (dest, wA, tA, wB, tB): dest = wA.T@x_tA + wB.T@x_tB
            specs = [
                (qT_re, 0, 0, 2, 1),
                (qT_im, 1, 0, 0, 1),
                (kT_re, 3, 2, 5, 3),
                (kT_im, 4, 2, 3, 3),
                ("v_re", 6, 4, 8, 5),
                ("v_im", 7, 4, 6, 5),
            ]
            for ncc in range(NCC):
                xt = work.tile([128, 24 * NCW], f32r, tag="xt")
                nc.sync.dma_start(
                    xt[:].rearrange("p (b f) -> p b f", f=NCW),
                    xpack_e[ncc].rearrange("b p f -> p b f"))

                def xblk(t, rc):
                    return xt[:, (t * 4 + rc) * NCW:(t * 4 + rc + 1) * NCW]

                def wblk(w, rc):
                    return wp[:, (w * 4 + rc) * 128:(w * 4 + rc + 1) * 128]

                for si, (dest, wA, tA, wB, tB) in enumerate(specs):
                    pj = psA.tile([128, NCW], f32,
                                  tag=("s_re" if si % 2 == 0 else "s_im"),
                                  name=f"pj_{ncc}_{si}")
                    for rc in range(4):
                        nc.tensor.matmul(pj[:], wblk(wA, rc), xblk(tA, rc),
                                         start=(rc == 0), stop=False)
                    for rc in range(4):
                        nc.tensor.matmul(pj[:], wblk(wB, rc), xblk(tB, rc),
                                         start=False, stop=(rc == 3))
                    cs = slice(ncc * NCW, (ncc + 1) * NCW)
                    if dest == "v_re":
                        # head h real part -> rows 0:64 of vT16_h
                        for h in (0, 1):
                            nc.vector.tensor_copy(
                                vT16_h[h][0:64, cs], pj[64 * h:64 * h + 64, :])
                    elif dest == "v_im":
                        for h in (0, 1):
                            nc.vector.tensor_copy(
                                vT16_h[h][64:128, cs], pj[64 * h:64 * h + 64, :])
                    elif dest is qT_im:
                        nc.vector.tensor_copy(dest[:, cs], pj[:])
                    elif dest is kT_im:
                        nc.scalar.copy(dest[:, cs], pj[:])
                        nc.scalar.mul(kT_imn[:, cs], pj[:], -1.0)
                    else:
                        nc.scalar.copy(dest[:, cs], pj[:])

            # ---- V transpose via PE: v16_h[n, re|im] fp16 ----
            v16_h = [pers.tile([128, NK], f16, tag=f"v16_h{h}",
                               name=f"v16_h{h}") for h in (0, 1)]
            for h in (0, 1):
                for nt in range(KT):
                    blk = slice(nt * 128, (nt + 1) * 128)
                    vt_ps = psA.tile([128, 128], f16, tag="s_im",
                                     name=f"vtp_{h}_{nt}")
                    nc.tensor.transpose(vt_ps[:], vT16_h[h][:, blk],
                                        ident16[:])
                    if (h + nt) % 2 == 0:
                        nc.vector.tensor_copy(v16_h[h][:, blk], vt_ps[:])
                    else:
                        nc.scalar.copy(v16_h[h][:, blk], vt_ps[:])

            # ---- output accumulators for W_O ----
            oT_re = pers.tile([128, NQ], f32r, tag="oT_re")
            oT_im = pers.tile([128, NQ], f32r, tag="oT_im")

            # ---- attention ----
            for qc in range(QC):
                qs = slice(qc * QCW, (qc + 1) * QCW)
                o_ps = [psA.tile([128, QCW], f32, tag=f"o{h}",
                                 name=f"o{h}_{qc}") for h in (0, 1)]
                rs_ps = [psA.tile([128, QCW], f32, tag=f"rs{h}",
                                  name=f"rs{h}_{qc}") for h in (0, 1)]
                for kt in range(KT):
                    ks = slice(kt * 128, (kt + 1) * 128)
                    s_re = psA.tile([128, 1024], f32, tag="s_re")
                    s_im = psA.tile([128, 1024], f32, tag="s_im")
                    for h in (0, 1):
                        hs = slice(64 * h, 64 * h + 64)
                        col = slice(h * 512, h * 512 + 512)
                        tp = (64 * h, 0)
                        nc.tensor.matmul(s_re[:, col], kT_re[hs, ks],
                                         qT_re[hs, qs], start=True,
                                         stop=False, tile_position=tp)
                        nc.tensor.matmul(s_re[:, col], kT_im[hs, ks],
                                         qT_im[hs, qs], start=False,
                                         stop=True, tile_position=tp)
                        nc.tensor.matmul(s_im[:, col], kT_re[hs, ks],
                                         qT_im[hs, qs], start=True,
                                         stop=False, tile_position=tp)
                        nc.tensor.matmul(s_im[:, col], kT_imn[hs, ks],
                                         qT_re[hs, qs], start=False,
                                         stop=True, tile_position=tp)
                    t16 = work.tile([128, 1024], f16, tag="t16")
                    nc.vector.tensor_copy(t16[:], s_re[:])
                    sqre = work.tile([128, 1024], f16, tag="sqre")
                    nc.vector.tensor_mul(sqre[:], t16[:], t16[:])
                    sqim = work.tile([128, 1024], f16, tag="sqim")
                    nc.scalar.square(sqim[:], s_im[:])
                    ssq = work.tile([128, 1024], f16, tag="ssq")
                    nc.vector.tensor_add(ssq[:], sqre[:], sqim[:])
                    lnu = work.tile([128, 1024], f16, tag="lnu")
                    nc.scalar.activation(lnu[:], ssq[:], AF.Ln)
                    mag = work.tile([128, 1024], f16, tag="mag")
                    nc.scalar.activation(mag[:], lnu[:], AF.Exp,
                                         bias=eb_mag[:], scale=0.5)
                    p = pwork.tile([128, 1024], f16, tag="p")
                    nc.scalar.activation(p[:], mag[:], AF.Exp, bias=eb_exp[:])
                    if DEBUG and qc == 0 and kt == 0:
                        dp = pers.tile([128, 1024], f32, tag="dbg_p00",
                                       name="dbg_p00")
                        nc.vector.tensor_copy(dp[:], p[:])
                        nc.sync.dma_start(dbg_es["d_p00"][:], dp[:])
                        ds = pers.tile([128, 1024], f32, tag="dbg_sre00",
                                       name="dbg_sre00")
                        nc.scalar.copy(ds[:], s_re[:])
                        nc.sync.dma_start(dbg_es["d_sre00"][:], ds[:])
                    for h in (0, 1):
                        col = slice(h * 512, h * 512 + 512)
                        vblk = v16_h[h][:, kt * 128:(kt + 1) * 128]
                        nc.tensor.matmul(o_ps[h][:, :], vblk, p[:, col],
                                         start=(kt == 0), stop=(kt == KT - 1))
                        nc.tensor.matmul(rs_ps[h][0:1, :], ones16[:],
                                         p[:, col],
                                         start=(kt == 0), stop=(kt == KT - 1))
                # normalize: oT = o / rowsum
                bc = psA.tile([128, 1024], f32, tag="s_re",
                              name=f"bc_{qc}")
                for h in (0, 1):
                    recip = work.tile([1, QCW], f32r, tag=f"recip{h}")
                    nc.vector.reciprocal(recip[:], rs_ps[h][0:1, :])
                    nc.tensor.matmul(bc[:, h * 512:h * 512 + 512],
                                     ones_row[:], recip[:],
                                     start=True, stop=True)
                bc_sb = work.tile([128, 1024], f32r, tag="bc_sb")
                nc.scalar.copy(bc_sb[:], bc[:])
                if DEBUG and qc == 0:
                    dr = pers.tile([128, 512], f32, tag="dbg_rs0",
                                   name="dbg_rs0")
                    nc.vector.tensor_copy(dr[:], rs_ps[0][:])
                    nc.sync.dma_start(dbg_es["d_rs0"][:], dr[:])
                for h in (0, 1):
                    for ri, dest in ((0, oT_re), (1, oT_im)):
                        rows = slice(64 * ri, 64 * ri + 64)
                        nc.vector.scalar_tensor_tensor(
                            dest[64 * h:64 * h + 64, qs],
                            o_ps[h][rows, :], 1.0,
                            bc_sb[rows, h * 512:h * 512 + 512],
                            ALU.mult, ALU.mult)

            if DEBUG:
                for nm, t in (("d_qT_re", qT_re), ("d_kT_re", kT_re),
                              ("d_kT_imn", kT_imn), ("d_vT16_h0", vT16_h[0]),
                              ("d_v16_h0", v16_h[0]), ("d_oT_re", oT_re)):
                    dd = pers.tile(list(t.shape), f32, tag=f"dbg_{nm}",
                                   name=f"dbg_{nm}")
                    nc.vector.tensor_copy(dd[:], t[:])
                    nc.sync.dma_start(dbg_es[nm][:], dd[:])

            # ---- W_O projection (partial over this core's 128 hd) ----
            for Rc in range(4):
                for qc in range(QC):
                    qs = slice(qc * QCW, (qc + 1) * QCW)
                    wo_re = psA.tile([128, QCW], f32, tag="o0",
                                     name=f"wore_{Rc}_{qc}")
                    wo_im = psA.tile([128, QCW], f32, tag="o1",
                                     name=f"woim_{Rc}_{qc}")

                    def wob(w):
                        return wop[:, w * 512 + Rc * 128:
                                   w * 512 + Rc * 128 + 128]

                    nc.tensor.matmul(wo_re[:], wob(0), oT_re[:, qs],
                                     start=True, stop=False)
                    nc.tensor.matmul(wo_re[:], wob(2), oT_im[:, qs],
                                     start=False, stop=True)
                    nc.tensor.matmul(wo_im[:], wob(1), oT_re[:, qs],
                                     start=True, stop=False)
                    nc.tensor.matmul(wo_im[:], wob(0), oT_im[:, qs],
                                     start=False, stop=True)
                    st_re = work.tile([128, QCW], f32, tag="st_re")
                    nc.scalar.copy(st_re[:], wo_re[:])
                    nc.sync.dma_start(
                        ore_e[Rc * 128:(Rc + 1) * 128, qs], st_re[:])
                    st_im = work.tile([128, QCW], f32, tag="st_im")
                    nc.vector.tensor_copy(st_im[:], wo_im[:])
                    nc.sync.dma_start(
                        oim_e[Rc * 128:(Rc + 1) * 128, qs], st_im[:])

    nc.finalize()
    return nc


def _get_nc():
    if "nc" not in _CACHE:
        _CACHE["nc"] = _build_nc()
    return _CACHE["nc"]


def _core_inputs(c, inputs):
    b = c // 4
    h0 = 2 * (c % 4)
    hs = slice(h0 * 64, h0 * 64 + 128)

    xpack = np.empty((NCC, 24, 128, NCW), np.float32)
    for t, name in enumerate(
            ("Q_real", "Q_imag", "K_real", "K_imag", "V_real", "V_imag")):
        xT = np.ascontiguousarray(inputs[name][b].T)          # (512, 2048)
        xpack[:, t * 4:(t + 1) * 4] = (
            xT.reshape(4, 128, NCC, NCW).transpose(2, 0, 1, 3))

    wq_r, wq_i = inputs["wq_r"][hs], inputs["wq_i"][hs]       # (128, 512)
    wk_r, wk_i = inputs["wk_r"][hs], inputs["wk_i"][hs]
    wv_r, wv_i = inputs["wv_r"][hs], inputs["wv_i"][hs]
    wlist = [wq_r, wq_i, -wq_i, wk_r, wk_i, -wk_i, wv_r, wv_i, -wv_i]
    arr36 = np.empty((36, 128, 128), np.float32)
    for wi, mat in enumerate(wlist):
        arr36[wi * 4:(wi + 1) * 4] = np.ascontiguousarray(mat.T).reshape(
            4, 128, 128)
    wpack = np.ascontiguousarray(arr36.transpose(1, 0, 2)).reshape(128, 36 * 128)

    wo_r_T = np.ascontiguousarray(inputs["wo_r"][:, hs].T)    # (128, 512)
    wo_i_T = np.ascontiguousarray(inputs["wo_i"][:, hs].T)
    wopack = np.concatenate([wo_r_T, wo_i_T, -wo_i_T], axis=1)
    wopack = np.ascontiguousarray(wopack)

    return {
        "xpack": xpack,
        "wpack": wpack,
        "wopack": wopack,
        "onesr": np.ones((1, 128), np.float32),
        "ident": np.eye(128, dtype=np.float16),
    }


def kernel(**inputs):
    from concourse.bass_utils import run_bass_kernel_spmd

    nc = _get_nc()
    in_maps = [_core_inputs(c, inputs) for c in range(NCORES)]
    res = run_bass_kernel_spmd(nc, in_maps, list(range(NCORES)))
    out = np.empty((B, NQ, R, 2), np.float32)
    for b in range(B):
        re = np.zeros((512, 2048), np.float64)
        im = np.zeros((512, 2048), np.float64)
        for c in range(b * 4, b * 4 + 4):
            re += res.results[c]["out_re"]
            im += res.results[c]["out_im"]
        out[b, :, :, 0] = re.T
        out[b, :, :, 1] = im.T
    return out
